# revision 12
# baseline (speedup 1.0000x reference)
"""Trainium2 Bass kernel for nn_AGTLayer (GAT-style additive-attention transformer layer).

Key algebraic fact exploited: softmax over j of (sl[i] + sr[j]) is independent
of sl (constant shift along the softmax axis), so the N x N attention matrix
collapses to a single weight vector per (batch, head):
    p[b,h,i,:] = softmax_j(sr[b,h,:])      (same for every query i)
    ctx[b,h,i,:] = sum_j p[b,h,j] fr[b,h,j,:]   (one vector per (b,h))
Hence fl / Wl / att_l never influence the output, and the layer reduces to:
    fr = h @ Wr.T
    sr[b,h,j] = leaky(fr[b,j,h*128:(h+1)*128]) . att_r
    w = exp(sr)  (values are O(1); no max subtraction needed)
    c[b,h,:] = (sum_j w[j] fr[b,j,head]) / sum_j w[j]
    fh[b,:] = concat_h(c[b,h,:]) @ Wf.T
    out = LayerNorm(h + fh[:,None,:]) * gamma + beta

Sharding: flatten (B,N) -> 8192 rows, 1024 rows per core (cores 2b, 2b+1 hold
batch b). Each core computes fr for its rows and partial softmax sums; a tiny
pairwise AllGather + on-chip add combines the halves; each core redundantly
computes fh for its batch and applies the LayerNorm epilogue to its rows.

Hardware notes baked in:
 - PSUM `start=True` clears the whole bank -> one accumulation group per bank.
 - ACT LUT reloads on function switch (~1.3us) -> Exp batched into one call.
 - PE HAM clock-gate: dummy matmul bursts keep the PE at 2.4GHz through the
   DMA-ingest window and the collective window.
 - hT is shipped in per-row-tile contiguous blocks so the first matmul only
   needs 256KB of hT + Wr instead of the full 2MB.
"""

import numpy as np
import ml_dtypes
from contextlib import ExitStack

import concourse.bass as bass
import concourse.mybir as mybir
import concourse.tile as tile
from concourse import bacc
from concourse.bass_utils import run_bass_kernel_spmd

AF = mybir.ActivationFunctionType
ALU = mybir.AluOpType
F32 = mybir.dt.float32
BF16 = mybir.dt.bfloat16

B, N, D, H, HD = 4, 2048, 1024, 8, 128
NCORES = 8
ROWS = (B * N) // NCORES  # 1024 rows per core
P = 128                   # partitions
KT = D // P               # 8 k-tiles
MT = ROWS // P            # 8 row-tiles per core
NB = 512                  # psum bank free-dim (f32)
LN_EPS = 1e-5
CCW = 1040                # AllGather payload width (bf16, 32B-aligned)


def _bcast_ap(ap, parts, free, dtype_ignored=None):
    return bass.AP(tensor=ap.tensor, offset=ap.offset, ap=[[0, parts], [1, free]])


def _build(apply_gb: bool):
    nc = bacc.Bacc(
        "TRN2",
        target_bir_lowering=False,
        debug=False,
        enable_asserts=False,
        num_devices=NCORES,
    )

    # hTm: h-shard transposed, blocked per row-tile so each row-tile's weights
    # load as one flat [128, 1024] DMA: hTm[mt*P + p, kt*P + j] = h[mt*128+j, kt*128+p]
    hTm = nc.dram_tensor("hTm", [MT * P, KT * P], BF16, kind="ExternalInput")
    hF = nc.dram_tensor("hF", [ROWS, D], BF16, kind="ExternalInput")
    WrT = nc.dram_tensor("WrT", [D, D], BF16, kind="ExternalInput")
    WfT = nc.dram_tensor("WfT", [D, D], BF16, kind="ExternalInput")
    attr = nc.dram_tensor("attr", [1, D], BF16, kind="ExternalInput")
    sel = nc.dram_tensor("sel", [NCORES, 1], BF16, kind="ExternalInput")
    out = nc.dram_tensor("out", [ROWS, D], F32, kind="ExternalOutput")
    if apply_gb:
        gam = nc.dram_tensor("gam", [1, D], F32, kind="ExternalInput")
        bet = nc.dram_tensor("bet", [1, D], F32, kind="ExternalInput")

    with tile.TileContext(nc) as tc, ExitStack() as ctx:
        const = ctx.enter_context(tc.tile_pool(name="const", bufs=1))
        work = ctx.enter_context(tc.tile_pool(name="work", bufs=3))
        ep = ctx.enter_context(tc.tile_pool(name="ep", bufs=3))
        eps_p = ctx.enter_context(tc.tile_pool(name="eps", bufs=4))
        frp = ctx.enter_context(tc.tile_pool(name="frp", bufs=2, space="PSUM"))
        accp = ctx.enter_context(tc.tile_pool(name="accp", bufs=1, space="PSUM"))
        sp = ctx.enter_context(tc.tile_pool(name="sp", bufs=1, space="PSUM"))
        wp = ctx.enter_context(tc.tile_pool(name="wp", bufs=1, space="PSUM"))
        dram = ctx.enter_context(tc.tile_pool(name="dram", bufs=1, space="DRAM"))

        # ---- tiny constants (fast) ----
        ones_m = const.tile([P, 1], BF16, tag="ones_m")
        nc.vector.memset(ones_m[:], 1.0)
        ones1 = const.tile([1, P], F32, tag="ones1")
        nc.vector.memset(ones1[:], 1.0)
        eps_sb = const.tile([P, 1], F32, tag="eps")
        nc.vector.memset(eps_sb[:], LN_EPS)
        warm_w = const.tile([P, NB], BF16, tag="warm_w")
        nc.vector.memset(warm_w[:], 0.0)

        # ---- PE warmup burst #1: runs while input DMAs stream in ----
        warm_ps = wp.tile([P, NB], F32, tag="warm")
        NWARM1, NWARM2 = 12, 26
        for i in range(NWARM1):
            nc.tensor.matmul(warm_ps[:], lhsT=warm_w[:, 0:P], rhs=warm_w[:],
                             start=(i == 0), stop=False)

        # ---- input loads, spread across engine DGE queues ----
        # critical-path first: wr[kt] + ht_m[0] feed the first matmuls
        wr_t, wf_t, h_t, ht_m = [], [], [], []
        for kt in range(KT):
            t = const.tile([P, D], BF16, tag=f"wr{kt}")
            nc.sync.dma_start(out=t[:], in_=WrT.ap()[kt * P:(kt + 1) * P, :])
            wr_t.append(t)
        for mt in range(MT):
            t = const.tile([P, KT * P], BF16, tag=f"htm{mt}")
            nc.scalar.dma_start(out=t[:], in_=hTm.ap()[mt * P:(mt + 1) * P, :])
            ht_m.append(t)
        for kt in range(KT):
            t = const.tile([P, D], BF16, tag=f"wf{kt}")
            nc.gpsimd.dma_start(out=t[:], in_=WfT.ap()[kt * P:(kt + 1) * P, :])
            wf_t.append(t)
        for mt in range(MT):
            t = const.tile([P, D], BF16, tag=f"h{mt}")
            nc.gpsimd.dma_start(out=t[:], in_=hF.ap()[mt * P:(mt + 1) * P, :])
            h_t.append(t)
        att_sb = const.tile([P, D], BF16, tag="att")
        nc.gpsimd.dma_start(out=att_sb[:], in_=_bcast_ap(attr.ap(), P, D))
        if apply_gb:
            gam_sb = const.tile([P, D], F32, tag="gam")
            nc.gpsimd.dma_start(out=gam_sb[:], in_=_bcast_ap(gam.ap(), P, D))
            bet_sb = const.tile([P, D], F32, tag="bet")
            nc.gpsimd.dma_start(out=bet_sb[:], in_=_bcast_ap(bet.ap(), P, D))

        # ---- main loop: fr matmul + leaky/att/reduce into sr_all ----
        sr_all = const.tile([P, MT, H], F32, tag="sr_all")
        frb_t = []
        for mt in range(MT):
            fr_ps = frp.tile([P, D], F32, tag="fr")
            for kt in range(KT):
                lw = ht_m[mt][:, kt * P:(kt + 1) * P]
                for nh in range(2):
                    nc.tensor.matmul(
                        fr_ps[:, nh * NB:(nh + 1) * NB],
                        lhsT=lw,
                        rhs=wr_t[kt][:, nh * NB:(nh + 1) * NB],
                        start=(kt == 0),
                        stop=(kt == KT - 1),
                    )
            frb = const.tile([P, D], BF16, tag=f"frb{mt}")
            nc.scalar.activation(out=frb[:], in_=fr_ps[:], func=AF.Copy)
            lky = work.tile([P, D], BF16, tag="lky")
            nc.scalar.activation(out=lky[:], in_=fr_ps[:], func=AF.Lrelu,
                                 alpha=0.01)
            t2 = work.tile([P, D], BF16, tag="t2")
            nc.vector.tensor_tensor(out=t2[:], in0=lky[:], in1=att_sb[:],
                                    op=ALU.mult)
            nc.vector.tensor_reduce(
                out=sr_all[:, mt, :],
                in_=t2[:].rearrange("p (h hd) -> p h hd", h=H),
                axis=mybir.AxisListType.X,
                op=ALU.add,
            )
            frb_t.append(frb)

        # ---- batched exp, then cpart = w.T @ fr (one group per PSUM bank) ----
        w_all = const.tile([P, MT, H], BF16, tag="w_all")
        nc.scalar.activation(out=w_all[:], in_=sr_all[:], func=AF.Exp)

        cs_ps = accp.tile([H, D], F32, tag="acc")
        s_ps = sp.tile([H, 8], F32, tag="s")
        for mt in range(MT):
            for nh in range(2):
                nc.tensor.matmul(
                    cs_ps[0:H, nh * NB:(nh + 1) * NB],
                    lhsT=w_all[:, mt, :],
                    rhs=frb_t[mt][:, nh * NB:(nh + 1) * NB],
                    start=(mt == 0),
                    stop=(mt == MT - 1),
                )
            nc.tensor.matmul(s_ps[0:H, 0:1], lhsT=w_all[:, mt, :],
                             rhs=ones_m[:], start=(mt == 0),
                             stop=(mt == MT - 1))

        # ---- PE warmup burst #2: keeps PE warm through the collective ----
        for i in range(NWARM2):
            nc.tensor.matmul(warm_ps[:], lhsT=warm_w[:, 0:P], rhs=warm_w[:],
                             start=False, stop=(i == NWARM2 - 1))
        warm_sb = const.tile([1, 1], BF16, tag="warm_sb")
        nc.vector.tensor_copy(out=warm_sb[:], in_=warm_ps[0:1, 0:1])

        # ---- pairwise AllGather of (diag blocks of cpart, s), add on-chip ----
        cs_sb = const.tile([H, D], BF16, tag="cs_sb")
        nc.vector.tensor_copy(out=cs_sb[:], in_=cs_ps[:])
        s_sb = const.tile([H, 1], BF16, tag="s_sb")
        nc.vector.tensor_copy(out=s_sb[:], in_=s_ps[:, 0:1])

        sel_sb = const.tile([NCORES, 1], BF16, tag="sel")
        nc.gpsimd.dma_start(out=sel_sb[:], in_=sel.ap())
        cc_in = dram.tile([1, CCW], BF16, tag="ccin")
        cc_ag = dram.tile([NCORES, CCW], BF16, tag="ccag")
        engs = [nc.gpsimd, nc.scalar, nc.sync]
        for hh in range(H):
            engs[hh % 3].dma_start(out=cc_in[0:1, hh * HD:(hh + 1) * HD],
                                   in_=cs_sb[hh:hh + 1, hh * HD:(hh + 1) * HD])
        nc.gpsimd.dma_start(out=cc_in[0:1, D:D + H], in_=s_sb[:])
        nc.scalar.dma_start(out=cc_in[0:1, D + H:D + H + 1], in_=warm_sb[:])
        nc.gpsimd.collective_compute(
            "AllGather",
            ALU.bypass,
            replica_groups=[list(range(NCORES))],
            ins=[cc_in[:].opt()],
            outs=[cc_ag[:].opt()],
        )
        ag_sb = const.tile([NCORES, CCW], BF16, tag="ag_sb")
        nc.gpsimd.dma_start(out=ag_sb[:], in_=cc_ag[:])

        # column-sum of the two gathered rows via PE (K=2 matmul)
        cs_tot = accp.tile([1, D], F32, tag="acc")  # reuses cs_ps slot
        for nh in range(2):
            nc.tensor.matmul(cs_tot[0:1, nh * NB:(nh + 1) * NB],
                             lhsT=sel_sb[:], rhs=ag_sb[:, nh * NB:(nh + 1) * NB],
                             start=True, stop=True)
        s_tot = sp.tile([1, H], F32, tag="s")  # reuses s_ps slot
        nc.tensor.matmul(s_tot[:], lhsT=sel_sb[:], rhs=ag_sb[:, D:D + H],
                         start=True, stop=True)

        # c = cpart / s per head; reshape c to [128, H] via a DRAM bounce
        cs_tot_sb = const.tile([1, D], BF16, tag="cs_tot_sb")
        nc.vector.tensor_copy(out=cs_tot_sb[:], in_=cs_tot[:])
        cs_dram = dram.tile([1, D], BF16, tag="csd")
        nc.scalar.dma_start(out=cs_dram[:], in_=cs_tot_sb[:])
        csum = const.tile([P, H], BF16, tag="csum")
        cbase = cs_dram[0:1, 0:1]
        nc.gpsimd.dma_start(
            out=csum[:],
            in_=bass.AP(tensor=cbase.tensor, offset=cbase.offset,
                        ap=[[1, P], [P, H]]),
        )
        ssum = const.tile([1, H], F32, tag="ssum")
        nc.vector.tensor_copy(out=ssum[:], in_=s_tot[:])
        rs = const.tile([1, H], F32, tag="rs")
        nc.vector.reciprocal(out=rs[:], in_=ssum[:])
        rsb_ps = sp.tile([P, H], F32, tag="s")  # reuses s slot
        nc.tensor.matmul(rsb_ps[:], lhsT=ones1[:], rhs=rs[:], start=True,
                         stop=True)
        cn = const.tile([P, H], BF16, tag="cn")
        nc.vector.tensor_tensor(out=cn[:], in0=csum[:], in1=rsb_ps[:],
                                op=ALU.mult)

        # fh = c @ Wf.T  (matvec over k-tiles; column h of cn is k-tile h)
        fh_ps = accp.tile([1, D], F32, tag="acc")
        for kt in range(KT):
            for nh in range(2):
                nc.tensor.matmul(
                    fh_ps[0:1, nh * NB:(nh + 1) * NB],
                    lhsT=cn[:, kt:kt + 1],
                    rhs=wf_t[kt][:, nh * NB:(nh + 1) * NB],
                    start=(kt == 0),
                    stop=(kt == KT - 1),
                )
        fh_dram = dram.tile([1, D], BF16, tag="fhd")
        fh_sb = const.tile([1, D], BF16, tag="fh_sb")
        nc.vector.tensor_copy(out=fh_sb[:], in_=fh_ps[:])
        nc.scalar.dma_start(out=fh_dram[:], in_=fh_sb[:])
        fhb = const.tile([P, D], BF16, tag="fhb")
        fd = fh_dram[0:1, :]
        nc.gpsimd.dma_start(
            out=fhb[:],
            in_=bass.AP(tensor=fd.tensor, offset=fd.offset, ap=[[0, P], [1, D]]),
        )

        # ---- epilogue: y = h + fh, LayerNorm over d, write out ----
        mv_all = eps_p.tile([P, MT, 2], F32, tag="mv_all")
        y_t = []
        for mt in range(MT):
            y = ep.tile([P, D], F32, tag=f"y{mt % 4}")
            eng = nc.vector if mt % 2 == 0 else nc.gpsimd
            eng.tensor_tensor(out=y[:], in0=h_t[mt][:], in1=fhb[:], op=ALU.add)
            st = eps_p.tile([P, 2, 6], F32, tag="st")
            nc.vector.bn_stats(out=st[:, 0, :], in_=y[:, 0:NB])
            nc.vector.bn_stats(out=st[:, 1, :], in_=y[:, NB:D])
            nc.vector.bn_aggr(out=mv_all[:, mt, :], in_=st[:])
            y_t.append(y)
        sd_all = eps_p.tile([P, MT], F32, tag="sd_all")
        nc.scalar.activation(out=sd_all[:], in_=mv_all[:, :, 1], func=AF.Sqrt,
                             bias=eps_sb[:])
        rstd_all = eps_p.tile([P, MT], F32, tag="rstd_all")
        nc.vector.reciprocal(out=rstd_all[:], in_=sd_all[:])
        nmr_all = eps_p.tile([P, MT], F32, tag="nmr_all")
        nc.vector.scalar_tensor_tensor(out=nmr_all[:], in0=mv_all[:, :, 0],
                                       scalar=-1.0, in1=rstd_all[:],
                                       op0=ALU.mult, op1=ALU.mult)
        for mt in range(MT):
            o = ep.tile([P, D], F32, tag="o")
            aeng = nc.gpsimd if mt % 2 == 0 else nc.scalar
            if mt % 2 == 0:
                aeng.tensor_scalar(out=o[:], in0=y_t[mt][:],
                                   scalar1=mv_all[:, mt, 0:1],
                                   scalar2=rstd_all[:, mt:mt + 1],
                                   op0=ALU.subtract, op1=ALU.mult)
            else:
                nc.scalar.activation(out=o[:], in_=y_t[mt][:],
                                     func=AF.Identity,
                                     scale=rstd_all[:, mt:mt + 1],
                                     bias=nmr_all[:, mt:mt + 1])
            if apply_gb:
                nc.gpsimd.tensor_tensor(out=o[:], in0=o[:], in1=gam_sb[:],
                                        op=ALU.mult)
                nc.gpsimd.tensor_tensor(out=o[:], in0=o[:], in1=bet_sb[:],
                                        op=ALU.add)
            eng = nc.sync if mt % 2 == 0 else nc.scalar
            eng.dma_start(out=out.ap()[mt * P:(mt + 1) * P, :], in_=o[:])

    nc.compile()
    return nc


_NC_CACHE = {}


def _get_nc(apply_gb: bool):
    if apply_gb not in _NC_CACHE:
        _NC_CACHE[apply_gb] = _build(apply_gb)
    return _NC_CACHE[apply_gb]


def _make_in_maps(h, Wr, att_r, Wf, ln_gamma, ln_beta, apply_gb):
    hf = np.ascontiguousarray(np.asarray(h, np.float32).reshape(B * N, D))
    WrT = np.ascontiguousarray(np.asarray(Wr, np.float32).T).astype(
        ml_dtypes.bfloat16)
    WfT = np.ascontiguousarray(np.asarray(Wf, np.float32).T).astype(
        ml_dtypes.bfloat16)
    at = np.tile(np.asarray(att_r, np.float32).reshape(1, HD), (1, H)).astype(
        ml_dtypes.bfloat16)
    in_maps = []
    for i in range(NCORES):
        sh = hf[i * ROWS:(i + 1) * ROWS]
        shT = sh.T.astype(ml_dtypes.bfloat16)          # [D, ROWS] = [kt*P+p, mt*P+j]
        hTm = np.ascontiguousarray(
            shT.reshape(KT, P, MT, P).transpose(2, 1, 0, 3).reshape(MT * P, KT * P))
        selv = np.zeros((NCORES, 1), ml_dtypes.bfloat16)
        selv[(i // 2) * 2, 0] = 1.0
        selv[(i // 2) * 2 + 1, 0] = 1.0
        m = {
            "hTm": hTm,
            "sel": selv,
            "hF": sh.astype(ml_dtypes.bfloat16),
            "WrT": WrT,
            "WfT": WfT,
            "attr": at,
        }
        if apply_gb:
            m["gam"] = np.asarray(ln_gamma, np.float32).reshape(1, D)
            m["bet"] = np.asarray(ln_beta, np.float32).reshape(1, D)
        in_maps.append(m)
    return in_maps


def _run(h, Wl, Wr, att_l, att_r, Wf, ln_gamma, ln_beta, trace=False):
    g = np.asarray(ln_gamma, np.float32)
    b = np.asarray(ln_beta, np.float32)
    apply_gb = not (np.all(g == 1.0) and np.all(b == 0.0))
    nc = _get_nc(apply_gb)
    in_maps = _make_in_maps(h, Wr, att_r, Wf, ln_gamma, ln_beta, apply_gb)
    res = run_bass_kernel_spmd(nc, in_maps, core_ids=list(range(NCORES)),
                               trace=trace)
    outs = [res.results[i]["out"] for i in range(NCORES)]
    full = np.concatenate(outs, axis=0).reshape(B, N, D).astype(np.float32)
    return full, res


def kernel(**inputs):
    out, _ = _run(**inputs)
    return out


# revision 13
# speedup vs baseline: 1.2175x; 1.2175x over previous
"""Trainium2 Bass kernel for nn_AGTLayer (GAT-style additive-attention transformer layer).

Key algebraic fact exploited: softmax over j of (sl[i] + sr[j]) is independent
of sl (constant shift along the softmax axis), so the N x N attention matrix
collapses to a single weight vector per (batch, head):
    p[b,h,i,:] = softmax_j(sr[b,h,:])      (same for every query i)
    ctx[b,h,i,:] = sum_j p[b,h,j] fr[b,h,j,:]   (one vector per (b,h))
Hence fl / Wl / att_l never influence the output, and the layer reduces to:
    fr = h @ Wr.T
    sr[b,h,j] = leaky(fr[b,j,h*128:(h+1)*128]) . att_r
    w = exp(sr)  (values are O(1); no max subtraction needed)
    c[b,h,:] = (sum_j w[j] fr[b,j,head]) / sum_j w[j]
    fh[b,:] = concat_h(c[b,h,:]) @ Wf.T
    out = LayerNorm(h + fh[:,None,:]) * gamma + beta

Sharding: flatten (B,N) -> 8192 rows, 1024 rows per core (cores 2b, 2b+1 hold
batch b). Each core computes fr for its rows and partial softmax sums; a tiny
pairwise AllGather + on-chip add combines the halves; each core redundantly
computes fh for its batch and applies the LayerNorm epilogue to its rows.

Hardware notes baked in:
 - PSUM `start=True` clears the whole bank -> one accumulation group per bank.
 - ACT LUT reloads on function switch (~1.3us) -> Exp batched into one call.
 - PE HAM clock-gate: dummy matmul bursts keep the PE at 2.4GHz through the
   DMA-ingest window and the collective window.
 - hT is shipped in per-row-tile contiguous blocks so the first matmul only
   needs 256KB of hT + Wr instead of the full 2MB.
"""

import numpy as np
import ml_dtypes
from contextlib import ExitStack

import concourse.bass as bass
import concourse.mybir as mybir
import concourse.tile as tile
from concourse import bacc
from concourse.bass_utils import run_bass_kernel_spmd

AF = mybir.ActivationFunctionType
ALU = mybir.AluOpType
F32 = mybir.dt.float32
BF16 = mybir.dt.bfloat16

B, N, D, H, HD = 4, 2048, 1024, 8, 128
NCORES = 8
ROWS = (B * N) // NCORES  # 1024 rows per core
P = 128                   # partitions
KT = D // P               # 8 k-tiles
MT = ROWS // P            # 8 row-tiles per core
NB = 512                  # psum bank free-dim (f32)
LN_EPS = 1e-5
CCW = 1040                # AllGather payload width (bf16, 32B-aligned)


def _bcast_ap(ap, parts, free, dtype_ignored=None):
    return bass.AP(tensor=ap.tensor, offset=ap.offset, ap=[[0, parts], [1, free]])


def _build(apply_gb: bool):
    nc = bacc.Bacc(
        "TRN2",
        target_bir_lowering=False,
        debug=False,
        enable_asserts=False,
        num_devices=NCORES,
    )

    # hTm: h-shard transposed, blocked per row-tile so each row-tile's weights
    # load as one flat [128, 1024] DMA: hTm[mt*P + p, kt*P + j] = h[mt*128+j, kt*128+p]
    hTm = nc.dram_tensor("hTm", [MT * P, KT * P], BF16, kind="ExternalInput")
    hF = nc.dram_tensor("hF", [ROWS, D], BF16, kind="ExternalInput")
    WrT = nc.dram_tensor("WrT", [D, D], BF16, kind="ExternalInput")
    WfT = nc.dram_tensor("WfT", [D, D], BF16, kind="ExternalInput")
    attr = nc.dram_tensor("attr", [1, D], BF16, kind="ExternalInput")
    out = nc.dram_tensor("out", [ROWS, D], F32, kind="ExternalOutput")
    if apply_gb:
        gam = nc.dram_tensor("gam", [1, D], F32, kind="ExternalInput")
        bet = nc.dram_tensor("bet", [1, D], F32, kind="ExternalInput")

    with tile.TileContext(nc) as tc, ExitStack() as ctx:
        const = ctx.enter_context(tc.tile_pool(name="const", bufs=1))
        work = ctx.enter_context(tc.tile_pool(name="work", bufs=3))
        ep = ctx.enter_context(tc.tile_pool(name="ep", bufs=3))
        eps_p = ctx.enter_context(tc.tile_pool(name="eps", bufs=4))
        frp = ctx.enter_context(tc.tile_pool(name="frp", bufs=2, space="PSUM"))
        accp = ctx.enter_context(tc.tile_pool(name="accp", bufs=1, space="PSUM"))
        sp = ctx.enter_context(tc.tile_pool(name="sp", bufs=1, space="PSUM"))
        wp = ctx.enter_context(tc.tile_pool(name="wp", bufs=1, space="PSUM"))
        dram = ctx.enter_context(tc.tile_pool(name="dram", bufs=1, space="DRAM"))

        # ---- tiny constants (fast) ----
        ones_m = const.tile([P, 1], BF16, tag="ones_m")
        nc.vector.memset(ones_m[:], 1.0)
        ones1 = const.tile([1, P], F32, tag="ones1")
        nc.vector.memset(ones1[:], 1.0)
        eps_sb = const.tile([P, 1], F32, tag="eps")
        nc.vector.memset(eps_sb[:], LN_EPS)
        warm_w = const.tile([P, NB], BF16, tag="warm_w")
        nc.vector.memset(warm_w[:], 0.0)

        # ---- PE warmup burst #1: runs while input DMAs stream in ----
        warm_ps = wp.tile([P, NB], F32, tag="warm")
        NWARM1, NWARM2 = 12, 26
        for i in range(NWARM1):
            nc.tensor.matmul(warm_ps[:], lhsT=warm_w[:, 0:P], rhs=warm_w[:],
                             start=(i == 0), stop=False)

        # ---- input loads, spread across engine DGE queues ----
        # critical-path first: wr[kt] + ht_m[0] feed the first matmuls
        att_sb = const.tile([P, D], BF16, tag="att")
        nc.sync.dma_start(out=att_sb[:], in_=_bcast_ap(attr.ap(), P, D))
        wr_t, wf_t, h_t, ht_m = [], [], [], []
        for kt in range(KT):
            t = const.tile([P, D], BF16, tag=f"wr{kt}")
            nc.sync.dma_start(out=t[:], in_=WrT.ap()[kt * P:(kt + 1) * P, :])
            wr_t.append(t)
        for mt in range(MT):
            t = const.tile([P, KT * P], BF16, tag=f"htm{mt}")
            nc.scalar.dma_start(out=t[:], in_=hTm.ap()[mt * P:(mt + 1) * P, :])
            ht_m.append(t)
        for kt in range(KT):
            t = const.tile([P, D], BF16, tag=f"wf{kt}")
            nc.gpsimd.dma_start(out=t[:], in_=WfT.ap()[kt * P:(kt + 1) * P, :])
            wf_t.append(t)
        for mt in range(MT):
            t = const.tile([P, D], BF16, tag=f"h{mt}")
            nc.gpsimd.dma_start(out=t[:], in_=hF.ap()[mt * P:(mt + 1) * P, :])
            h_t.append(t)
        if apply_gb:
            gam_sb = const.tile([P, D], F32, tag="gam")
            nc.gpsimd.dma_start(out=gam_sb[:], in_=_bcast_ap(gam.ap(), P, D))
            bet_sb = const.tile([P, D], F32, tag="bet")
            nc.gpsimd.dma_start(out=bet_sb[:], in_=_bcast_ap(bet.ap(), P, D))

        # ---- main loop: fr matmul + leaky/att/reduce into sr_all ----
        sr_all = const.tile([P, MT, H], F32, tag="sr_all")
        frb_t = []
        for mt in range(MT):
            fr_ps = frp.tile([P, D], F32, tag="fr")
            for kt in range(KT):
                lw = ht_m[mt][:, kt * P:(kt + 1) * P]
                for nh in range(2):
                    nc.tensor.matmul(
                        fr_ps[:, nh * NB:(nh + 1) * NB],
                        lhsT=lw,
                        rhs=wr_t[kt][:, nh * NB:(nh + 1) * NB],
                        start=(kt == 0),
                        stop=(kt == KT - 1),
                    )
            frb = const.tile([P, D], BF16, tag=f"frb{mt}")
            nc.scalar.activation(out=frb[:], in_=fr_ps[:], func=AF.Copy)
            lky = work.tile([P, D], BF16, tag="lky")
            nc.scalar.activation(out=lky[:], in_=fr_ps[:], func=AF.Lrelu,
                                 alpha=0.01)
            t2 = work.tile([P, D], BF16, tag="t2")
            nc.vector.tensor_tensor(out=t2[:], in0=lky[:], in1=att_sb[:],
                                    op=ALU.mult)
            nc.vector.tensor_reduce(
                out=sr_all[:, mt, :],
                in_=t2[:].rearrange("p (h hd) -> p h hd", h=H),
                axis=mybir.AxisListType.X,
                op=ALU.add,
            )
            frb_t.append(frb)

        # ---- batched exp, then cpart = w.T @ fr (one group per PSUM bank) ----
        w_all = const.tile([P, MT, H], BF16, tag="w_all")
        nc.scalar.activation(out=w_all[:], in_=sr_all[:], func=AF.Exp)

        cs_ps = accp.tile([H, D], F32, tag="acc")
        s_ps = sp.tile([H, 8], F32, tag="s")
        for mt in range(MT):
            for nh in range(2):
                nc.tensor.matmul(
                    cs_ps[0:H, nh * NB:(nh + 1) * NB],
                    lhsT=w_all[:, mt, :],
                    rhs=frb_t[mt][:, nh * NB:(nh + 1) * NB],
                    start=(mt == 0),
                    stop=(mt == MT - 1),
                )
            nc.tensor.matmul(s_ps[0:H, 0:1], lhsT=w_all[:, mt, :],
                             rhs=ones_m[:], start=(mt == 0),
                             stop=(mt == MT - 1))

        # ---- PE warmup burst #2: keeps PE warm through the collective ----
        for i in range(NWARM2):
            nc.tensor.matmul(warm_ps[:], lhsT=warm_w[:, 0:P], rhs=warm_w[:],
                             start=False, stop=(i == NWARM2 - 1))
        warm_sb = const.tile([1, 1], BF16, tag="warm_sb")
        nc.vector.tensor_copy(out=warm_sb[:], in_=warm_ps[0:1, 0:1])

        # ---- pairwise AllGather of (diag blocks of cpart, s), add on-chip ----
        cs_sb = const.tile([H, D], BF16, tag="cs_sb")
        nc.vector.tensor_copy(out=cs_sb[:], in_=cs_ps[:])
        s_sb = const.tile([H, 1], BF16, tag="s_sb")
        nc.vector.tensor_copy(out=s_sb[:], in_=s_ps[:, 0:1])

        cc_in = dram.tile([1, CCW], BF16, tag="ccin")
        cc_out = dram.tile([1, CCW], BF16, tag="ccout")
        engs = [nc.gpsimd, nc.scalar, nc.sync]
        for hh in range(H):
            cdst = cc_in[0:1, hh:hh + 1]
            engs[hh % 3].dma_start(
                out=bass.AP(tensor=cdst.tensor, offset=cdst.offset,
                            ap=[[0, 1], [H, HD]]),
                in_=cs_sb[hh:hh + 1, hh * HD:(hh + 1) * HD])
        nc.gpsimd.dma_start(out=cc_in[0:1, D:D + H], in_=s_sb[:])
        nc.scalar.dma_start(out=cc_in[0:1, D + H:D + H + 1], in_=warm_sb[:])
        nc.gpsimd.collective_compute(
            "AllReduce",
            ALU.add,
            replica_groups=[[0, 1], [2, 3], [4, 5], [6, 7]],
            ins=[cc_in[:].opt()],
            outs=[cc_out[:].opt()],
        )
        # c = cpart / s per head; reshape c to [128, H] straight from DRAM
        csum = const.tile([P, H], BF16, tag="csum")
        cbase = cc_out[0:1, 0:1]
        nc.gpsimd.dma_start(
            out=csum[:],
            in_=bass.AP(tensor=cbase.tensor, offset=cbase.offset,
                        ap=[[H, P], [1, H]]),
        )
        ssum = const.tile([1, H], BF16, tag="ssum")
        nc.scalar.dma_start(out=ssum[:], in_=cc_out[0:1, D:D + H])
        rs = const.tile([1, H], F32, tag="rs")
        nc.vector.reciprocal(out=rs[:], in_=ssum[:])
        rsb_ps = sp.tile([P, H], F32, tag="s")  # reuses s slot
        nc.tensor.matmul(rsb_ps[:], lhsT=ones1[:], rhs=rs[:], start=True,
                         stop=True)
        cn = const.tile([P, H], BF16, tag="cn")
        nc.vector.tensor_tensor(out=cn[:], in0=csum[:], in1=rsb_ps[:],
                                op=ALU.mult)

        # fh = c @ Wf.T  (matvec over k-tiles; column h of cn is k-tile h)
        fh_ps = accp.tile([1, D], F32, tag="acc")
        for kt in range(KT):
            for nh in range(2):
                nc.tensor.matmul(
                    fh_ps[0:1, nh * NB:(nh + 1) * NB],
                    lhsT=cn[:, kt:kt + 1],
                    rhs=wf_t[kt][:, nh * NB:(nh + 1) * NB],
                    start=(kt == 0),
                    stop=(kt == KT - 1),
                )
        fh_dram = dram.tile([1, D], BF16, tag="fhd")
        fh_sb = const.tile([1, D], BF16, tag="fh_sb")
        nc.vector.tensor_copy(out=fh_sb[:], in_=fh_ps[:])
        nc.scalar.dma_start(out=fh_dram[:], in_=fh_sb[:])
        fhb = const.tile([P, D], BF16, tag="fhb")
        fd = fh_dram[0:1, :]
        nc.gpsimd.dma_start(
            out=fhb[:],
            in_=bass.AP(tensor=fd.tensor, offset=fd.offset, ap=[[0, P], [1, D]]),
        )

        # ---- epilogue: y = h + fh, LayerNorm over d, write out ----
        mv_all = eps_p.tile([P, MT, 2], F32, tag="mv_all")
        y_t = []
        for mt in range(MT):
            y = ep.tile([P, D], F32, tag=f"y{mt % 4}")
            eng = nc.vector if mt % 2 == 0 else nc.gpsimd
            eng.tensor_tensor(out=y[:], in0=h_t[mt][:], in1=fhb[:], op=ALU.add)
            st = eps_p.tile([P, 2, 6], F32, tag="st")
            nc.vector.bn_stats(out=st[:, 0, :], in_=y[:, 0:NB])
            nc.vector.bn_stats(out=st[:, 1, :], in_=y[:, NB:D])
            nc.vector.bn_aggr(out=mv_all[:, mt, :], in_=st[:])
            y_t.append(y)
        sd_all = eps_p.tile([P, MT], F32, tag="sd_all")
        nc.scalar.activation(out=sd_all[:], in_=mv_all[:, :, 1], func=AF.Sqrt,
                             bias=eps_sb[:])
        rstd_all = eps_p.tile([P, MT], F32, tag="rstd_all")
        nc.vector.reciprocal(out=rstd_all[:], in_=sd_all[:])
        nmr_all = eps_p.tile([P, MT], F32, tag="nmr_all")
        nc.vector.scalar_tensor_tensor(out=nmr_all[:], in0=mv_all[:, :, 0],
                                       scalar=-1.0, in1=rstd_all[:],
                                       op0=ALU.mult, op1=ALU.mult)
        for mt in range(MT):
            o = ep.tile([P, D], F32, tag="o")
            if mt % 2 == 0:
                nc.vector.tensor_scalar(out=o[:], in0=y_t[mt][:],
                                        scalar1=mv_all[:, mt, 0:1],
                                        scalar2=rstd_all[:, mt:mt + 1],
                                        op0=ALU.subtract, op1=ALU.mult)
            else:
                nc.scalar.activation(out=o[:], in_=y_t[mt][:],
                                     func=AF.Identity,
                                     scale=rstd_all[:, mt:mt + 1],
                                     bias=nmr_all[:, mt:mt + 1])
            if apply_gb:
                nc.gpsimd.tensor_tensor(out=o[:], in0=o[:], in1=gam_sb[:],
                                        op=ALU.mult)
                nc.gpsimd.tensor_tensor(out=o[:], in0=o[:], in1=bet_sb[:],
                                        op=ALU.add)
            eng = nc.sync if mt % 2 == 0 else nc.scalar
            eng.dma_start(out=out.ap()[mt * P:(mt + 1) * P, :], in_=o[:])

    nc.compile()
    return nc


_NC_CACHE = {}


def _get_nc(apply_gb: bool):
    if apply_gb not in _NC_CACHE:
        _NC_CACHE[apply_gb] = _build(apply_gb)
    return _NC_CACHE[apply_gb]


def _make_in_maps(h, Wr, att_r, Wf, ln_gamma, ln_beta, apply_gb):
    hf = np.ascontiguousarray(np.asarray(h, np.float32).reshape(B * N, D))
    WrT = np.ascontiguousarray(np.asarray(Wr, np.float32).T).astype(
        ml_dtypes.bfloat16)
    WfT = np.ascontiguousarray(np.asarray(Wf, np.float32).T).astype(
        ml_dtypes.bfloat16)
    at = np.tile(np.asarray(att_r, np.float32).reshape(1, HD), (1, H)).astype(
        ml_dtypes.bfloat16)
    in_maps = []
    for i in range(NCORES):
        sh = hf[i * ROWS:(i + 1) * ROWS]
        shT = sh.T.astype(ml_dtypes.bfloat16)          # [D, ROWS] = [kt*P+p, mt*P+j]
        hTm = np.ascontiguousarray(
            shT.reshape(KT, P, MT, P).transpose(2, 1, 0, 3).reshape(MT * P, KT * P))
        m = {
            "hTm": hTm,
            "hF": sh.astype(ml_dtypes.bfloat16),
            "WrT": WrT,
            "WfT": WfT,
            "attr": at,
        }
        if apply_gb:
            m["gam"] = np.asarray(ln_gamma, np.float32).reshape(1, D)
            m["bet"] = np.asarray(ln_beta, np.float32).reshape(1, D)
        in_maps.append(m)
    return in_maps


def _run(h, Wl, Wr, att_l, att_r, Wf, ln_gamma, ln_beta, trace=False):
    g = np.asarray(ln_gamma, np.float32)
    b = np.asarray(ln_beta, np.float32)
    apply_gb = not (np.all(g == 1.0) and np.all(b == 0.0))
    nc = _get_nc(apply_gb)
    in_maps = _make_in_maps(h, Wr, att_r, Wf, ln_gamma, ln_beta, apply_gb)
    res = run_bass_kernel_spmd(nc, in_maps, core_ids=list(range(NCORES)),
                               trace=trace)
    outs = [res.results[i]["out"] for i in range(NCORES)]
    full = np.concatenate(outs, axis=0).reshape(B, N, D).astype(np.float32)
    return full, res


def kernel(**inputs):
    out, _ = _run(**inputs)
    return out


# revision 14
# speedup vs baseline: 1.4404x; 1.1831x over previous
"""Trainium2 Bass kernel for nn_AGTLayer (GAT-style additive-attention transformer layer).

Key algebraic fact exploited: softmax over j of (sl[i] + sr[j]) is independent
of sl (constant shift along the softmax axis), so the N x N attention matrix
collapses to a single weight vector per (batch, head):
    p[b,h,i,:] = softmax_j(sr[b,h,:])      (same for every query i)
    ctx[b,h,i,:] = sum_j p[b,h,j] fr[b,h,j,:]   (one vector per (b,h))
Hence fl / Wl / att_l never influence the output, and the layer reduces to:
    fr = h @ Wr.T
    sr[b,h,j] = leaky(fr[b,j,h*128:(h+1)*128]) . att_r
    w = exp(sr)  (values are O(1); no max subtraction needed)
    c[b,h,:] = (sum_j w[j] fr[b,j,head]) / sum_j w[j]
    fh[b,:] = concat_h(c[b,h,:]) @ Wf.T
    out = LayerNorm(h + fh[:,None,:]) * gamma + beta

Sharding: flatten (B,N) -> 8192 rows, 1024 rows per core (cores 2b, 2b+1 hold
batch b). Each core computes fr for its rows and partial softmax sums; a tiny
pairwise AllGather + on-chip add combines the halves; each core redundantly
computes fh for its batch and applies the LayerNorm epilogue to its rows.

Hardware notes baked in:
 - PSUM `start=True` clears the whole bank -> one accumulation group per bank.
 - ACT LUT reloads on function switch (~1.3us) -> Exp batched into one call.
 - PE HAM clock-gate: dummy matmul bursts keep the PE at 2.4GHz through the
   DMA-ingest window and the collective window.
 - hT is shipped in per-row-tile contiguous blocks so the first matmul only
   needs 256KB of hT + Wr instead of the full 2MB.
"""

import numpy as np
import ml_dtypes
from contextlib import ExitStack

import concourse.bass as bass
import concourse.mybir as mybir
import concourse.tile as tile
from concourse import bacc
from concourse.bass_utils import run_bass_kernel_spmd

AF = mybir.ActivationFunctionType
ALU = mybir.AluOpType
F32 = mybir.dt.float32
BF16 = mybir.dt.bfloat16

B, N, D, H, HD = 4, 2048, 1024, 8, 128
NCORES = 8
ROWS = (B * N) // NCORES  # 1024 rows per core
P = 128                   # partitions
KT = D // P               # 8 k-tiles
MT = ROWS // P            # 8 row-tiles per core
NB = 512                  # psum bank free-dim (f32)
LN_EPS = 1e-5
CCW = 1040                # AllGather payload width (bf16, 32B-aligned)


def _bcast_ap(ap, parts, free, dtype_ignored=None):
    return bass.AP(tensor=ap.tensor, offset=ap.offset, ap=[[0, parts], [1, free]])


def _build(apply_gb: bool):
    nc = bacc.Bacc(
        "TRN2",
        target_bir_lowering=False,
        debug=False,
        enable_asserts=False,
        num_devices=NCORES,
    )

    # hTm: h-shard transposed, blocked per row-tile so each row-tile's weights
    # load as one flat [128, 1024] DMA: hTm[mt*P + p, kt*P + j] = h[mt*128+j, kt*128+p]
    hTm = nc.dram_tensor("hTm", [MT * P, KT * P], BF16, kind="ExternalInput")
    hF = nc.dram_tensor("hF", [ROWS, D], BF16, kind="ExternalInput")
    WrT = nc.dram_tensor("WrT", [D, D], BF16, kind="ExternalInput")
    WfT = nc.dram_tensor("WfT", [D, D], BF16, kind="ExternalInput")
    attr = nc.dram_tensor("attr", [1, D], BF16, kind="ExternalInput")
    out = nc.dram_tensor("out", [ROWS, D], F32, kind="ExternalOutput")
    if apply_gb:
        gam = nc.dram_tensor("gam", [1, D], F32, kind="ExternalInput")
        bet = nc.dram_tensor("bet", [1, D], F32, kind="ExternalInput")

    with tile.TileContext(nc) as tc, ExitStack() as ctx:
        const = ctx.enter_context(tc.tile_pool(name="const", bufs=1))
        work = ctx.enter_context(tc.tile_pool(name="work", bufs=3))
        ep = ctx.enter_context(tc.tile_pool(name="ep", bufs=3))
        eps_p = ctx.enter_context(tc.tile_pool(name="eps", bufs=4))
        frp = ctx.enter_context(tc.tile_pool(name="frp", bufs=2, space="PSUM"))
        accp = ctx.enter_context(tc.tile_pool(name="accp", bufs=1, space="PSUM"))
        sp = ctx.enter_context(tc.tile_pool(name="sp", bufs=1, space="PSUM"))
        wp = ctx.enter_context(tc.tile_pool(name="wp", bufs=1, space="PSUM"))
        dram = ctx.enter_context(tc.tile_pool(name="dram", bufs=1, space="DRAM"))

        # ---- tiny constants (fast) ----
        ones_m = const.tile([P, 1], BF16, tag="ones_m")
        nc.vector.memset(ones_m[:], 1.0)
        ones1 = const.tile([1, P], F32, tag="ones1")
        nc.vector.memset(ones1[:], 1.0)
        eps_sb = const.tile([P, 1], F32, tag="eps")
        nc.vector.memset(eps_sb[:], LN_EPS)
        warm_w = const.tile([P, NB], BF16, tag="warm_w")
        nc.vector.memset(warm_w[:], 0.0)

        # ---- PE warmup burst #1: runs while input DMAs stream in ----
        warm_ps = wp.tile([P, NB], F32, tag="warm")
        NWARM1, NWARM2 = 12, 48
        for i in range(NWARM1):
            nc.tensor.matmul(warm_ps[:], lhsT=warm_w[:, 0:P], rhs=warm_w[:],
                             start=(i == 0), stop=False)

        # ---- input loads, spread across engine DGE queues ----
        # critical-path first: wr[kt] + ht_m[0] feed the first matmuls
        att_sb = const.tile([P, D], BF16, tag="att")
        nc.sync.dma_start(out=att_sb[:], in_=_bcast_ap(attr.ap(), P, D))
        wr_t, wf_t, h_t, ht_m = [], [], [], []
        for kt in range(KT):
            t = const.tile([P, D], BF16, tag=f"wr{kt}")
            eng = nc.sync if kt % 2 == 0 else nc.scalar
            eng.dma_start(out=t[:], in_=WrT.ap()[kt * P:(kt + 1) * P, :])
            wr_t.append(t)
        for mt in range(MT):
            t = const.tile([P, KT * P], BF16, tag=f"htm{mt}")
            eng = nc.scalar if mt % 2 == 0 else nc.sync
            eng.dma_start(out=t[:], in_=hTm.ap()[mt * P:(mt + 1) * P, :])
            ht_m.append(t)
        for kt in range(KT):
            t = const.tile([P, D], BF16, tag=f"wf{kt}")
            eng = nc.sync if kt % 2 == 0 else nc.scalar
            eng.dma_start(out=t[:], in_=WfT.ap()[kt * P:(kt + 1) * P, :])
            wf_t.append(t)
        for mt in range(MT):
            t = const.tile([P, D], BF16, tag=f"h{mt}")
            eng = nc.scalar if mt % 2 == 0 else nc.sync
            eng.dma_start(out=t[:], in_=hF.ap()[mt * P:(mt + 1) * P, :])
            h_t.append(t)
        if apply_gb:
            gam_sb = const.tile([P, D], F32, tag="gam")
            nc.sync.dma_start(out=gam_sb[:], in_=_bcast_ap(gam.ap(), P, D))
            bet_sb = const.tile([P, D], F32, tag="bet")
            nc.sync.dma_start(out=bet_sb[:], in_=_bcast_ap(bet.ap(), P, D))

        # ---- main loop: fr matmul + leaky/att/reduce into sr_all ----
        sr_all = const.tile([P, MT, H], F32, tag="sr_all")
        frb_t = []
        for mt in range(MT):
            fr_ps = frp.tile([P, D], F32, tag="fr")
            for kt in range(KT):
                lw = ht_m[mt][:, kt * P:(kt + 1) * P]
                for nh in range(2):
                    nc.tensor.matmul(
                        fr_ps[:, nh * NB:(nh + 1) * NB],
                        lhsT=lw,
                        rhs=wr_t[kt][:, nh * NB:(nh + 1) * NB],
                        start=(kt == 0),
                        stop=(kt == KT - 1),
                    )
            frb = const.tile([P, D], BF16, tag=f"frb{mt}")
            nc.scalar.activation(out=frb[:], in_=fr_ps[:], func=AF.Copy)
            lky = work.tile([P, D], BF16, tag="lky")
            nc.scalar.activation(out=lky[:], in_=fr_ps[:], func=AF.Lrelu,
                                 alpha=0.01)
            t2 = work.tile([P, D], BF16, tag="t2")
            nc.vector.tensor_tensor(out=t2[:], in0=lky[:], in1=att_sb[:],
                                    op=ALU.mult)
            nc.vector.tensor_reduce(
                out=sr_all[:, mt, :],
                in_=t2[:].rearrange("p (h hd) -> p h hd", h=H),
                axis=mybir.AxisListType.X,
                op=ALU.add,
            )
            frb_t.append(frb)

        # ---- batched exp, then cpart = w.T @ fr (one group per PSUM bank) ----
        w_all = const.tile([P, MT, H], BF16, tag="w_all")
        nc.scalar.activation(out=w_all[:], in_=sr_all[:], func=AF.Exp)

        cs_ps = accp.tile([H, D], F32, tag="acc")
        s_ps = sp.tile([H, 8], F32, tag="s")
        for mt in range(MT):
            for nh in range(2):
                nc.tensor.matmul(
                    cs_ps[0:H, nh * NB:(nh + 1) * NB],
                    lhsT=w_all[:, mt, :],
                    rhs=frb_t[mt][:, nh * NB:(nh + 1) * NB],
                    start=(mt == 0),
                    stop=(mt == MT - 1),
                )
            nc.tensor.matmul(s_ps[0:H, 0:1], lhsT=w_all[:, mt, :],
                             rhs=ones_m[:], start=(mt == 0),
                             stop=(mt == MT - 1))

        # ---- PE warmup burst #2: keeps PE warm through the collective ----
        for i in range(NWARM2):
            nc.tensor.matmul(warm_ps[:], lhsT=warm_w[:, 0:P], rhs=warm_w[:],
                             start=False, stop=(i == NWARM2 - 1))
        warm_sb = const.tile([1, 1], BF16, tag="warm_sb")
        nc.vector.tensor_copy(out=warm_sb[:], in_=warm_ps[0:1, 0:1])

        # ---- pairwise AllGather of (diag blocks of cpart, s), add on-chip ----
        cs_sb = const.tile([H, D], BF16, tag="cs_sb")
        nc.vector.tensor_copy(out=cs_sb[:], in_=cs_ps[:])
        s_sb = const.tile([H, 1], BF16, tag="s_sb")
        nc.vector.tensor_copy(out=s_sb[:], in_=s_ps[:, 0:1])

        cc_in = dram.tile([1, CCW], BF16, tag="ccin")
        cc_out = dram.tile([1, CCW], BF16, tag="ccout")
        engs = [nc.scalar, nc.sync]
        for hh in range(H):
            engs[hh % 2].dma_start(out=cc_in[0:1, hh * HD:(hh + 1) * HD],
                                   in_=cs_sb[hh:hh + 1, hh * HD:(hh + 1) * HD])
        nc.scalar.dma_start(out=cc_in[0:1, D:D + H], in_=s_sb[:])
        nc.sync.dma_start(out=cc_in[0:1, D + H:D + H + 1], in_=warm_sb[:])
        nc.gpsimd.collective_compute(
            "AllReduce",
            ALU.add,
            replica_groups=[[0, 1], [2, 3], [4, 5], [6, 7]],
            ins=[cc_in[:].opt()],
            outs=[cc_out[:].opt()],
        )
        # c = cpart / s per head; reshape c to [128, H] straight from DRAM
        csum = const.tile([P, H], BF16, tag="csum")
        for hh in range(H):
            cbase = cc_out[0:1, hh * HD:(hh + 1) * HD]
            engs[hh % 2].dma_start(
                out=csum[:, hh:hh + 1],
                in_=bass.AP(tensor=cbase.tensor, offset=cbase.offset,
                            ap=[[1, P], [1, 1]]),
            )
        ssum = const.tile([1, H], BF16, tag="ssum")
        nc.scalar.dma_start(out=ssum[:], in_=cc_out[0:1, D:D + H])
        rs = const.tile([1, H], F32, tag="rs")
        nc.vector.reciprocal(out=rs[:], in_=ssum[:])
        rsb_ps = sp.tile([P, H], F32, tag="s")  # reuses s slot
        nc.tensor.matmul(rsb_ps[:], lhsT=ones1[:], rhs=rs[:], start=True,
                         stop=True)
        cn = const.tile([P, H], BF16, tag="cn")
        nc.vector.tensor_tensor(out=cn[:], in0=csum[:], in1=rsb_ps[:],
                                op=ALU.mult)

        # fh = c @ Wf.T  (matvec over k-tiles; column h of cn is k-tile h)
        fh_ps = accp.tile([1, D], F32, tag="acc")
        for kt in range(KT):
            for nh in range(2):
                nc.tensor.matmul(
                    fh_ps[0:1, nh * NB:(nh + 1) * NB],
                    lhsT=cn[:, kt:kt + 1],
                    rhs=wf_t[kt][:, nh * NB:(nh + 1) * NB],
                    start=(kt == 0),
                    stop=(kt == KT - 1),
                )
        fh_dram = dram.tile([1, D], BF16, tag="fhd")
        fh_sb = const.tile([1, D], BF16, tag="fh_sb")
        nc.vector.tensor_copy(out=fh_sb[:], in_=fh_ps[:])
        nc.scalar.dma_start(out=fh_dram[:], in_=fh_sb[:])
        fhb = const.tile([P, D], BF16, tag="fhb")
        fd = fh_dram[0:1, :]
        nc.sync.dma_start(
            out=fhb[:],
            in_=bass.AP(tensor=fd.tensor, offset=fd.offset, ap=[[0, P], [1, D]]),
        )

        # ---- epilogue: y = h + fh, LayerNorm over d, write out ----
        mv_all = eps_p.tile([P, MT, 2], F32, tag="mv_all")
        y_t = []
        for mt in range(MT):
            y = ep.tile([P, D], BF16, tag=f"y{mt % 4}")
            eng = nc.vector if mt % 2 == 0 else nc.gpsimd
            eng.tensor_tensor(out=y[:], in0=h_t[mt][:], in1=fhb[:], op=ALU.add)
            st = eps_p.tile([P, 2, 6], F32, tag="st")
            nc.vector.bn_stats(out=st[:, 0, :], in_=y[:, 0:NB])
            nc.vector.bn_stats(out=st[:, 1, :], in_=y[:, NB:D])
            nc.vector.bn_aggr(out=mv_all[:, mt, :], in_=st[:])
            y_t.append(y)
        sd_all = eps_p.tile([P, MT], F32, tag="sd_all")
        nc.scalar.activation(out=sd_all[:], in_=mv_all[:, :, 1], func=AF.Sqrt,
                             bias=eps_sb[:])
        rstd_all = eps_p.tile([P, MT], F32, tag="rstd_all")
        nc.vector.reciprocal(out=rstd_all[:], in_=sd_all[:])
        nmr_all = eps_p.tile([P, MT], F32, tag="nmr_all")
        nc.vector.scalar_tensor_tensor(out=nmr_all[:], in0=mv_all[:, :, 0],
                                       scalar=-1.0, in1=rstd_all[:],
                                       op0=ALU.mult, op1=ALU.mult)
        for mt in range(MT):
            o = ep.tile([P, D], F32, tag="o")
            nc.scalar.activation(out=o[:], in_=y_t[mt][:],
                                 func=AF.Identity,
                                 scale=rstd_all[:, mt:mt + 1],
                                 bias=nmr_all[:, mt:mt + 1])
            if apply_gb:
                nc.gpsimd.tensor_tensor(out=o[:], in0=o[:], in1=gam_sb[:],
                                        op=ALU.mult)
                nc.gpsimd.tensor_tensor(out=o[:], in0=o[:], in1=bet_sb[:],
                                        op=ALU.add)
            eng = nc.sync if mt % 2 == 0 else nc.scalar
            eng.dma_start(out=out.ap()[mt * P:(mt + 1) * P, :], in_=o[:])

    nc.compile()
    return nc


_NC_CACHE = {}


def _get_nc(apply_gb: bool):
    if apply_gb not in _NC_CACHE:
        _NC_CACHE[apply_gb] = _build(apply_gb)
    return _NC_CACHE[apply_gb]


def _make_in_maps(h, Wr, att_r, Wf, ln_gamma, ln_beta, apply_gb):
    hf = np.ascontiguousarray(np.asarray(h, np.float32).reshape(B * N, D))
    WrT = np.ascontiguousarray(np.asarray(Wr, np.float32).T).astype(
        ml_dtypes.bfloat16)
    WfT = np.ascontiguousarray(np.asarray(Wf, np.float32).T).astype(
        ml_dtypes.bfloat16)
    at = np.tile(np.asarray(att_r, np.float32).reshape(1, HD), (1, H)).astype(
        ml_dtypes.bfloat16)
    in_maps = []
    for i in range(NCORES):
        sh = hf[i * ROWS:(i + 1) * ROWS]
        shT = sh.T.astype(ml_dtypes.bfloat16)          # [D, ROWS] = [kt*P+p, mt*P+j]
        hTm = np.ascontiguousarray(
            shT.reshape(KT, P, MT, P).transpose(2, 1, 0, 3).reshape(MT * P, KT * P))
        m = {
            "hTm": hTm,
            "hF": sh.astype(ml_dtypes.bfloat16),
            "WrT": WrT,
            "WfT": WfT,
            "attr": at,
        }
        if apply_gb:
            m["gam"] = np.asarray(ln_gamma, np.float32).reshape(1, D)
            m["bet"] = np.asarray(ln_beta, np.float32).reshape(1, D)
        in_maps.append(m)
    return in_maps


def _run(h, Wl, Wr, att_l, att_r, Wf, ln_gamma, ln_beta, trace=False):
    g = np.asarray(ln_gamma, np.float32)
    b = np.asarray(ln_beta, np.float32)
    apply_gb = not (np.all(g == 1.0) and np.all(b == 0.0))
    nc = _get_nc(apply_gb)
    in_maps = _make_in_maps(h, Wr, att_r, Wf, ln_gamma, ln_beta, apply_gb)
    res = run_bass_kernel_spmd(nc, in_maps, core_ids=list(range(NCORES)),
                               trace=trace)
    outs = [res.results[i]["out"] for i in range(NCORES)]
    full = np.concatenate(outs, axis=0).reshape(B, N, D).astype(np.float32)
    return full, res


def kernel(**inputs):
    out, _ = _run(**inputs)
    return out


# revision 15
# speedup vs baseline: 1.4520x; 1.0081x over previous
"""Trainium2 Bass kernel for nn_AGTLayer (GAT-style additive-attention transformer layer).

Key algebraic fact exploited: softmax over j of (sl[i] + sr[j]) is independent
of sl (constant shift along the softmax axis), so the N x N attention matrix
collapses to a single weight vector per (batch, head):
    p[b,h,i,:] = softmax_j(sr[b,h,:])      (same for every query i)
    ctx[b,h,i,:] = sum_j p[b,h,j] fr[b,h,j,:]   (one vector per (b,h))
Hence fl / Wl / att_l never influence the output, and the layer reduces to:
    fr = h @ Wr.T
    sr[b,h,j] = leaky(fr[b,j,h*128:(h+1)*128]) . att_r
    w = exp(sr)  (values are O(1); no max subtraction needed)
    c[b,h,:] = (sum_j w[j] fr[b,j,head]) / sum_j w[j]
    fh[b,:] = concat_h(c[b,h,:]) @ Wf.T
    out = LayerNorm(h + fh[:,None,:]) * gamma + beta

Sharding: flatten (B,N) -> 8192 rows, 1024 rows per core (cores 2b, 2b+1 hold
batch b). Each core computes fr for its rows and partial softmax sums; a tiny
pairwise AllGather + on-chip add combines the halves; each core redundantly
computes fh for its batch and applies the LayerNorm epilogue to its rows.

Hardware notes baked in:
 - PSUM `start=True` clears the whole bank -> one accumulation group per bank.
 - ACT LUT reloads on function switch (~1.3us) -> Exp batched into one call.
 - PE HAM clock-gate: dummy matmul bursts keep the PE at 2.4GHz through the
   DMA-ingest window and the collective window.
 - hT is shipped in per-row-tile contiguous blocks so the first matmul only
   needs 256KB of hT + Wr instead of the full 2MB.
"""

import numpy as np
import ml_dtypes
from contextlib import ExitStack

import concourse.bass as bass
import concourse.mybir as mybir
import concourse.tile as tile
from concourse import bacc
from concourse.bass_utils import run_bass_kernel_spmd

AF = mybir.ActivationFunctionType
ALU = mybir.AluOpType
F32 = mybir.dt.float32
BF16 = mybir.dt.bfloat16

B, N, D, H, HD = 4, 2048, 1024, 8, 128
NCORES = 8
ROWS = (B * N) // NCORES  # 1024 rows per core
P = 128                   # partitions
KT = D // P               # 8 k-tiles
MT = ROWS // P            # 8 row-tiles per core
NB = 512                  # psum bank free-dim (f32)
LN_EPS = 1e-5
CCW = 1040                # AllGather payload width (bf16, 32B-aligned)


def _bcast_ap(ap, parts, free, dtype_ignored=None):
    return bass.AP(tensor=ap.tensor, offset=ap.offset, ap=[[0, parts], [1, free]])


def _build(apply_gb: bool):
    nc = bacc.Bacc(
        "TRN2",
        target_bir_lowering=False,
        debug=False,
        enable_asserts=False,
        num_devices=NCORES,
    )

    # hTm: h-shard transposed, blocked per row-tile so each row-tile's weights
    # load as one flat [128, 1024] DMA: hTm[mt*P + p, kt*P + j] = h[mt*128+j, kt*128+p]
    hTm = nc.dram_tensor("hTm", [MT * P, KT * P], BF16, kind="ExternalInput")
    hF = nc.dram_tensor("hF", [ROWS, D], BF16, kind="ExternalInput")
    WrT = nc.dram_tensor("WrT", [D, D], BF16, kind="ExternalInput")
    WfT = nc.dram_tensor("WfT", [D, D], BF16, kind="ExternalInput")
    attr = nc.dram_tensor("attr", [1, D], BF16, kind="ExternalInput")
    out = nc.dram_tensor("out", [ROWS, D], F32, kind="ExternalOutput")
    if apply_gb:
        gam = nc.dram_tensor("gam", [1, D], F32, kind="ExternalInput")
        bet = nc.dram_tensor("bet", [1, D], F32, kind="ExternalInput")

    with tile.TileContext(nc) as tc, ExitStack() as ctx:
        const = ctx.enter_context(tc.tile_pool(name="const", bufs=1))
        work = ctx.enter_context(tc.tile_pool(name="work", bufs=3))
        ep = ctx.enter_context(tc.tile_pool(name="ep", bufs=3))
        eps_p = ctx.enter_context(tc.tile_pool(name="eps", bufs=4))
        frp = ctx.enter_context(tc.tile_pool(name="frp", bufs=2, space="PSUM"))
        accp = ctx.enter_context(tc.tile_pool(name="accp", bufs=1, space="PSUM"))
        sp = ctx.enter_context(tc.tile_pool(name="sp", bufs=1, space="PSUM"))
        wp = ctx.enter_context(tc.tile_pool(name="wp", bufs=1, space="PSUM"))
        dram = ctx.enter_context(tc.tile_pool(name="dram", bufs=1, space="DRAM"))

        # ---- tiny constants (fast) ----
        ones_m = const.tile([P, 1], BF16, tag="ones_m")
        nc.vector.memset(ones_m[:], 1.0)
        ones1 = const.tile([1, P], F32, tag="ones1")
        nc.vector.memset(ones1[:], 1.0)
        eps_sb = const.tile([P, 1], F32, tag="eps")
        nc.vector.memset(eps_sb[:], LN_EPS)
        warm_w = const.tile([P, NB], BF16, tag="warm_w")
        nc.vector.memset(warm_w[:], 0.0)

        # ---- PE warmup burst #1: runs while input DMAs stream in ----
        warm_ps = wp.tile([P, NB], F32, tag="warm")
        NWARM1, NWARM2 = 12, 100
        for i in range(NWARM1):
            nc.tensor.matmul(warm_ps[:], lhsT=warm_w[:, 0:P], rhs=warm_w[:],
                             start=(i == 0), stop=False)

        # ---- input loads, spread across engine DGE queues ----
        # critical-path first: wr[kt] + ht_m[0] feed the first matmuls
        att_sb = const.tile([P, D], BF16, tag="att")
        nc.sync.dma_start(out=att_sb[:], in_=_bcast_ap(attr.ap(), P, D))
        wr_t, wf_t, h_t, ht_m = [], [], [], []
        for kt in range(KT):
            t = const.tile([P, D], BF16, tag=f"wr{kt}")
            eng = nc.sync if kt % 2 == 0 else nc.scalar
            eng.dma_start(out=t[:], in_=WrT.ap()[kt * P:(kt + 1) * P, :])
            wr_t.append(t)
        for mt in range(MT):
            t = const.tile([P, KT * P], BF16, tag=f"htm{mt}")
            ht_m.append(t)
        for mt in range(MT):
            eng = nc.scalar if mt % 2 == 0 else nc.sync
            eng.dma_start(out=ht_m[mt][:], in_=hTm.ap()[mt * P:(mt + 1) * P, :])
        for kt in range(KT):
            t = const.tile([P, D], BF16, tag=f"wf{kt}")
            eng = nc.sync if kt % 2 == 0 else nc.scalar
            eng.dma_start(out=t[:], in_=WfT.ap()[kt * P:(kt + 1) * P, :])
            wf_t.append(t)
        for mt in range(MT):
            t = const.tile([P, D], BF16, tag=f"h{mt}")
            eng = nc.scalar if mt % 2 == 0 else nc.sync
            eng.dma_start(out=t[:], in_=hF.ap()[mt * P:(mt + 1) * P, :])
            h_t.append(t)
        if apply_gb:
            gam_sb = const.tile([P, D], F32, tag="gam")
            nc.sync.dma_start(out=gam_sb[:], in_=_bcast_ap(gam.ap(), P, D))
            bet_sb = const.tile([P, D], F32, tag="bet")
            nc.sync.dma_start(out=bet_sb[:], in_=_bcast_ap(bet.ap(), P, D))

        # ---- main loop: fr matmul + leaky/att/reduce into sr_all ----
        sr_all = const.tile([P, MT, H], F32, tag="sr_all")
        frb_t = []
        for mt in range(MT):
            fr_ps = frp.tile([P, D], F32, tag="fr")
            for kt in range(KT):
                lw = ht_m[mt][:, kt * P:(kt + 1) * P]
                for nh in range(2):
                    nc.tensor.matmul(
                        fr_ps[:, nh * NB:(nh + 1) * NB],
                        lhsT=lw,
                        rhs=wr_t[kt][:, nh * NB:(nh + 1) * NB],
                        start=(kt == 0),
                        stop=(kt == KT - 1),
                    )
            frb = const.tile([P, D], BF16, tag=f"frb{mt}")
            nc.scalar.activation(out=frb[:], in_=fr_ps[:], func=AF.Copy)
            lky = work.tile([P, D], BF16, tag="lky")
            nc.scalar.activation(out=lky[:], in_=fr_ps[:], func=AF.Lrelu,
                                 alpha=0.01)
            t2 = work.tile([P, D], BF16, tag="t2")
            nc.vector.tensor_tensor(out=t2[:], in0=lky[:], in1=att_sb[:],
                                    op=ALU.mult)
            nc.vector.tensor_reduce(
                out=sr_all[:, mt, :],
                in_=t2[:].rearrange("p (h hd) -> p h hd", h=H),
                axis=mybir.AxisListType.X,
                op=ALU.add,
            )
            frb_t.append(frb)

        # ---- batched exp, then cpart = w.T @ fr (one group per PSUM bank) ----
        w_all = const.tile([P, MT, H], BF16, tag="w_all")
        nc.scalar.activation(out=w_all[:], in_=sr_all[:], func=AF.Exp)

        cs_ps = accp.tile([H, D], F32, tag="acc")
        s_ps = sp.tile([H, 8], F32, tag="s")
        for mt in range(MT):
            for nh in range(2):
                nc.tensor.matmul(
                    cs_ps[0:H, nh * NB:(nh + 1) * NB],
                    lhsT=w_all[:, mt, :],
                    rhs=frb_t[mt][:, nh * NB:(nh + 1) * NB],
                    start=(mt == 0),
                    stop=(mt == MT - 1),
                )
            nc.tensor.matmul(s_ps[0:H, 0:1], lhsT=w_all[:, mt, :],
                             rhs=ones_m[:], start=(mt == 0),
                             stop=(mt == MT - 1))

        # ---- PE warmup burst #2: keeps PE warm through the collective ----
        for i in range(NWARM2):
            nc.tensor.matmul(warm_ps[:], lhsT=warm_w[:, 0:P], rhs=warm_w[:],
                             start=False, stop=(i == NWARM2 - 1))
        warm_sb = const.tile([1, 1], BF16, tag="warm_sb")
        nc.vector.tensor_copy(out=warm_sb[:], in_=warm_ps[0:1, 0:1])

        # ---- pairwise AllGather of (diag blocks of cpart, s), add on-chip ----
        cs_sb = const.tile([H, D], BF16, tag="cs_sb")
        nc.vector.tensor_copy(out=cs_sb[:], in_=cs_ps[:])
        s_sb = const.tile([H, 1], BF16, tag="s_sb")
        nc.vector.tensor_copy(out=s_sb[:], in_=s_ps[:, 0:1])

        cc_in = dram.tile([1, CCW], BF16, tag="ccin")
        cc_out = dram.tile([1, CCW], BF16, tag="ccout")
        engs = [nc.scalar, nc.sync]
        for hh in range(H):
            engs[hh % 2].dma_start(out=cc_in[0:1, hh * HD:(hh + 1) * HD],
                                   in_=cs_sb[hh:hh + 1, hh * HD:(hh + 1) * HD])
        nc.scalar.dma_start(out=cc_in[0:1, D:D + H], in_=s_sb[:])
        nc.sync.dma_start(out=cc_in[0:1, D + H:D + H + 1], in_=warm_sb[:])
        nc.gpsimd.collective_compute(
            "AllReduce",
            ALU.add,
            replica_groups=[[0, 1], [2, 3], [4, 5], [6, 7]],
            ins=[cc_in[:].opt()],
            outs=[cc_out[:].opt()],
        )
        # c = cpart / s per head; reshape c to [128, H] straight from DRAM
        csum = const.tile([P, H], BF16, tag="csum")
        for hh in range(H):
            cbase = cc_out[0:1, hh * HD:(hh + 1) * HD]
            engs[hh % 2].dma_start(
                out=csum[:, hh:hh + 1],
                in_=bass.AP(tensor=cbase.tensor, offset=cbase.offset,
                            ap=[[1, P], [1, 1]]),
            )
        ssum = const.tile([1, H], BF16, tag="ssum")
        nc.scalar.dma_start(out=ssum[:], in_=cc_out[0:1, D:D + H])
        rs = const.tile([1, H], F32, tag="rs")
        nc.vector.reciprocal(out=rs[:], in_=ssum[:])
        rsb_ps = sp.tile([P, H], F32, tag="s")  # reuses s slot
        nc.tensor.matmul(rsb_ps[:], lhsT=ones1[:], rhs=rs[:], start=True,
                         stop=True)
        cn = const.tile([P, H], BF16, tag="cn")
        nc.vector.tensor_tensor(out=cn[:], in0=csum[:], in1=rsb_ps[:],
                                op=ALU.mult)

        # fh = c @ Wf.T  (matvec over k-tiles; column h of cn is k-tile h)
        fh_ps = accp.tile([1, D], F32, tag="acc")
        for kt in range(KT):
            for nh in range(2):
                nc.tensor.matmul(
                    fh_ps[0:1, nh * NB:(nh + 1) * NB],
                    lhsT=cn[:, kt:kt + 1],
                    rhs=wf_t[kt][:, nh * NB:(nh + 1) * NB],
                    start=(kt == 0),
                    stop=(kt == KT - 1),
                )
        fh_dram = dram.tile([1, D], BF16, tag="fhd")
        fh_sb = const.tile([1, D], BF16, tag="fh_sb")
        nc.vector.tensor_copy(out=fh_sb[:], in_=fh_ps[:])
        nc.scalar.dma_start(out=fh_dram[:], in_=fh_sb[:])
        fhb = const.tile([P, D], BF16, tag="fhb")
        fd = fh_dram[0:1, :]
        nc.sync.dma_start(
            out=fhb[:],
            in_=bass.AP(tensor=fd.tensor, offset=fd.offset, ap=[[0, P], [1, D]]),
        )

        # ---- epilogue: y = h + fh, LayerNorm over d, write out ----
        mv_all = eps_p.tile([P, MT, 2], F32, tag="mv_all")
        y_t = []
        for mt in range(MT):
            y = ep.tile([P, D], BF16, tag=f"y{mt % 4}")
            nc.vector.tensor_tensor(out=y[:], in0=h_t[mt][:], in1=fhb[:],
                                    op=ALU.add)
            st = eps_p.tile([P, 2, 6], F32, tag="st")
            nc.vector.bn_stats(out=st[:, 0, :], in_=y[:, 0:NB])
            nc.vector.bn_stats(out=st[:, 1, :], in_=y[:, NB:D])
            nc.vector.bn_aggr(out=mv_all[:, mt, :], in_=st[:])
            y_t.append(y)
        sd_all = eps_p.tile([P, MT], F32, tag="sd_all")
        nc.scalar.activation(out=sd_all[:], in_=mv_all[:, :, 1], func=AF.Sqrt,
                             bias=eps_sb[:])
        rstd_all = eps_p.tile([P, MT], F32, tag="rstd_all")
        nc.vector.reciprocal(out=rstd_all[:], in_=sd_all[:])
        nmr_all = eps_p.tile([P, MT], F32, tag="nmr_all")
        nc.vector.scalar_tensor_tensor(out=nmr_all[:], in0=mv_all[:, :, 0],
                                       scalar=-1.0, in1=rstd_all[:],
                                       op0=ALU.mult, op1=ALU.mult)
        for mt in range(MT):
            o = ep.tile([P, D], F32, tag="o")
            nc.scalar.activation(out=o[:], in_=y_t[mt][:],
                                 func=AF.Identity,
                                 scale=rstd_all[:, mt:mt + 1],
                                 bias=nmr_all[:, mt:mt + 1])
            if apply_gb:
                nc.gpsimd.tensor_tensor(out=o[:], in0=o[:], in1=gam_sb[:],
                                        op=ALU.mult)
                nc.gpsimd.tensor_tensor(out=o[:], in0=o[:], in1=bet_sb[:],
                                        op=ALU.add)
            eng = nc.sync if mt % 2 == 0 else nc.scalar
            eng.dma_start(out=out.ap()[mt * P:(mt + 1) * P, :], in_=o[:])

    nc.compile()
    return nc


_NC_CACHE = {}


def _get_nc(apply_gb: bool):
    if apply_gb not in _NC_CACHE:
        _NC_CACHE[apply_gb] = _build(apply_gb)
    return _NC_CACHE[apply_gb]


def _make_in_maps(h, Wr, att_r, Wf, ln_gamma, ln_beta, apply_gb):
    hf = np.ascontiguousarray(np.asarray(h, np.float32).reshape(B * N, D))
    WrT = np.ascontiguousarray(np.asarray(Wr, np.float32).T).astype(
        ml_dtypes.bfloat16)
    WfT = np.ascontiguousarray(np.asarray(Wf, np.float32).T).astype(
        ml_dtypes.bfloat16)
    at = np.tile(np.asarray(att_r, np.float32).reshape(1, HD), (1, H)).astype(
        ml_dtypes.bfloat16)
    in_maps = []
    for i in range(NCORES):
        sh = hf[i * ROWS:(i + 1) * ROWS]
        shT = sh.T.astype(ml_dtypes.bfloat16)          # [D, ROWS] = [kt*P+p, mt*P+j]
        hTm = np.ascontiguousarray(
            shT.reshape(KT, P, MT, P).transpose(2, 1, 0, 3).reshape(MT * P, KT * P))
        m = {
            "hTm": hTm,
            "hF": sh.astype(ml_dtypes.bfloat16),
            "WrT": WrT,
            "WfT": WfT,
            "attr": at,
        }
        if apply_gb:
            m["gam"] = np.asarray(ln_gamma, np.float32).reshape(1, D)
            m["bet"] = np.asarray(ln_beta, np.float32).reshape(1, D)
        in_maps.append(m)
    return in_maps


def _run(h, Wl, Wr, att_l, att_r, Wf, ln_gamma, ln_beta, trace=False):
    g = np.asarray(ln_gamma, np.float32)
    b = np.asarray(ln_beta, np.float32)
    apply_gb = not (np.all(g == 1.0) and np.all(b == 0.0))
    nc = _get_nc(apply_gb)
    in_maps = _make_in_maps(h, Wr, att_r, Wf, ln_gamma, ln_beta, apply_gb)
    res = run_bass_kernel_spmd(nc, in_maps, core_ids=list(range(NCORES)),
                               trace=trace)
    outs = [res.results[i]["out"] for i in range(NCORES)]
    full = np.concatenate(outs, axis=0).reshape(B, N, D).astype(np.float32)
    return full, res


def kernel(**inputs):
    out, _ = _run(**inputs)
    return out
